# revision 9
# baseline (speedup 1.0000x reference)
"""Polyphase 2x upsample (scatter into one of 4 phases per batch) + circular
3x3 binomial blur, distributed over 8 TRN2 NeuronCores (data-parallel over
batch: 2 batches per core).

Math: with phase p per batch, r = p % 2, c = p // 2, the reference scatters
x[i,j] to y1[2i+r, 2j+c] (zeros elsewhere) and then blurs with
outer([1,2,1],[1,2,1])/16 under circular padding. The output decomposes into
4 parity classes (all indices mod 128):
  out[2i+r,   2j+c]   = x[i,j] / 4
  out[2i+r,   2k+1+c] = (x[i,k] + x[i,k+1]) / 8          (k+1 mod 64)
  out[2i+1+r, 2j+c]   = (x[i,j] + x[i+1,j]) / 8          (i+1 mod 64)
  out[2i+1+r, 2k+1+c] = (x[i,k]+x[i,k+1]+x[i+1,k]+x[i+1,k+1]) / 16
i.e. out = circular_shift_{r,c}(phase0_output). Pure shifted-add + scale:
no matmul, memory-bound.

Phase handling under SPMD (one NEFF for all 8 cores): the phase-0 output is
computed with fully static access patterns (parity-interleaved in SBUF).
The circular column shift by c is applied by a 2-piece copy pass whose
destination offsets (c and (127+c)%128) are runtime register values loaded
from a tiny per-core int32 input. The circular row shift by r is folded into
the output DMA's DRAM row offsets (r, 64+r, (127+r)%128): fixed shapes,
dynamic starts. Everything stays on-device; the host only computes 5 small
ints per batch. Dynamic-offset instructions permanently consume ~2 registers
on their issuing engine (no dedup), so they are kept rare and spread across
engines (DVE/ACT for the rotation copies; SP/Act/Pool DGE for the stores).
"""

import sys

for _p in ("/opt/trn_rl_repo",):
    if _p not in sys.path:
        sys.path.insert(0, _p)

import numpy as np

B, C, N = 16, 256, 64
M = 2 * N
NCORES = 8
NB = B // NCORES  # batches per core

_NC_CACHE = None


def _build_nc():
    import concourse.bacc as bacc
    import concourse.bass as bass
    import concourse.mybir as mybir
    import concourse.tile as tile

    f32 = mybir.dt.float32
    i32 = mybir.dt.int32
    add = mybir.AluOpType.add
    ds = bass.ds

    # Bacc (not plain Bass): its finalize() runs generate_event_semaphores,
    # which splits multi-wait instructions — this walrus build allows at most
    # one attached semaphore wait per instruction, and Tile emits several.
    nc = bacc.Bacc("TRN2", target_bir_lowering=False, debug=False, num_devices=NCORES)
    inp = nc.dram_tensor("inp", [NB, C, N, N], f32, kind="ExternalInput")
    offs = nc.dram_tensor("offs", [1, 16], i32, kind="ExternalInput")
    out = nc.dram_tensor("out", [NB, C, M, M], f32, kind="ExternalOutput")

    with tile.TileContext(nc) as tc:
        with (
            tc.tile_pool(name="offp", bufs=1) as offp,
            tc.tile_pool(name="xp", bufs=2) as xp,
            tc.tile_pool(name="tp", bufs=1) as tp,
            tc.tile_pool(name="op", bufs=1) as op,
            tc.tile_pool(name="rp", bufs=2) as rp,
        ):
            offs_t = offp.tile([1, 16], i32)
            nc.sync.dma_start(offs_t[:, :], offs[:, :])

            # per batch: [cR, cW, rA, rB, rC] at offs[0, 8*b + k].
            # Each dynamic-AP use burns registers on every engine of the
            # value's engine set, so load each value only on the engine(s)
            # that consume it (separate loads per engine).
            ET = mybir.EngineType
            ranges = {
                "cR": (0, 1),    # c
                "cW": (0, 127),  # (127 + c) % 128
                "rA": (0, 1),    # r
                "rB": (64, 65),  # 64 + r
                "rC": (0, 127),  # (127 + r) % 128
            }
            engmap = {
                "cR": (ET.DVE, ET.Activation),
                "cW": (ET.DVE, ET.Activation),
                "rA": (ET.SP,),
                "rB": (ET.Activation,),
                "rC": (ET.Pool,),
            }
            names = ("cR", "cW", "rA", "rB", "rC")
            val = {}
            for b in range(NB):
                for k, name in enumerate(names):
                    lo, hi = ranges[name]
                    for eng in engmap[name]:
                        # skip_runtime_bounds_check: the InstSeqAssert the
                        # software check emits faults this runtime
                        val[(b, name, eng)] = nc.values_load(
                            offs_t[0:1, 8 * b + k : 8 * b + k + 1],
                            engines=[eng],
                            min_val=lo,
                            max_val=hi,
                            skip_runtime_bounds_check=True,
                        )

            rot_toggle = 0
            for b in range(NB):
                rA = val[(b, "rA", ET.SP)]
                rB = val[(b, "rB", ET.Activation)]
                rC = val[(b, "rC", ET.Pool)]
                for h in range(C // 128):
                    chs = slice(128 * h, 128 * (h + 1))
                    # x: [128 ch, 64, 64]
                    x = xp.tile([128, N, N], f32, tag="x")
                    nc.sync.dma_start(x[:, :, :], inp[b, chs])

                    t8 = tp.tile([128, N, N], f32, tag="t8")
                    nc.scalar.mul(t8[:, :, :], x[:, :, :], 0.125)
                    t16 = tp.tile([128, N, N], f32, tag="t16")
                    nc.scalar.mul(t16[:, :, :], x[:, :, :], 0.0625)
                    # S[i,k] = (x[i,k] + x[i,k+1 mod 64]) / 16
                    S = tp.tile([128, N, N], f32, tag="S")
                    nc.vector.tensor_tensor(
                        S[:, :, 0:63], t16[:, :, 0:63], t16[:, :, 1:64], add
                    )
                    nc.vector.tensor_tensor(
                        S[:, :, 63:64], t16[:, :, 63:64], t16[:, :, 0:1], add
                    )

                    out3 = out[b, chs]  # [128 ch, 128, 128] DRAM view
                    for q in range(2):
                        i0 = 32 * q  # first input row of this chunk
                        # phase-0 half-image: local row 2i' = A/H of input row
                        # i0+i', local row 2i'+1 = V/D of pair (i0+i', i0+i'+1)
                        o = op.tile([128, 64, M], f32, tag="o")
                        # A sites: even rows, even cols
                        nc.scalar.mul(
                            o[:, 0:64:2, 0:128:2], x[:, i0 : i0 + 32, :], 0.25
                        )
                        # H sites: even rows, odd cols; pairs (k, k+1 mod 64)
                        nc.vector.tensor_tensor(
                            o[:, 0:64:2, 1:126:2],
                            t8[:, i0 : i0 + 32, 0:63],
                            t8[:, i0 : i0 + 32, 1:64],
                            add,
                        )
                        nc.vector.tensor_tensor(
                            o[:, 0:64:2, 127:128],
                            t8[:, i0 : i0 + 32, 63:64],
                            t8[:, i0 : i0 + 32, 0:1],
                            add,
                        )
                        # V sites: odd rows, even cols; pairs (i, i+1 mod 64)
                        if q == 0:
                            nc.vector.tensor_tensor(
                                o[:, 1:64:2, 0:128:2],
                                t8[:, 0:32, :],
                                t8[:, 1:33, :],
                                add,
                            )
                        else:
                            nc.vector.tensor_tensor(
                                o[:, 1:63:2, 0:128:2],
                                t8[:, 32:63, :],
                                t8[:, 33:64, :],
                                add,
                            )
                            nc.vector.tensor_tensor(
                                o[:, 63:64, 0:128:2],
                                t8[:, 63:64, :],
                                t8[:, 0:1, :],
                                add,
                            )
                        # D sites: odd rows, odd cols; S pairs (i, i+1 mod 64)
                        if q == 0:
                            nc.vector.tensor_tensor(
                                o[:, 1:64:2, 1:128:2],
                                S[:, 0:32, :],
                                S[:, 1:33, :],
                                add,
                            )
                        else:
                            nc.vector.tensor_tensor(
                                o[:, 1:63:2, 1:128:2],
                                S[:, 32:63, :],
                                S[:, 33:64, :],
                                add,
                            )
                            nc.vector.tensor_tensor(
                                o[:, 63:64, 1:128:2],
                                S[:, 63:64, :],
                                S[:, 0:1, :],
                                add,
                            )
                        # circular column shift by c: 2-piece copy,
                        # alternating engine to balance load
                        orot = rp.tile([128, 64, M], f32, tag="orot")
                        if rot_toggle % 2 == 0:
                            nc.vector.tensor_copy(
                                orot[:, :, ds(val[(b, "cR", ET.DVE)], 127)],
                                o[:, :, 0:127],
                            )
                        else:
                            nc.scalar.copy(
                                orot[:, :, ds(val[(b, "cR", ET.Activation)], 127)],
                                o[:, :, 0:127],
                            )
                        # wrap column always on DVE (tiny)
                        nc.vector.tensor_copy(
                            orot[:, :, ds(val[(b, "cW", ET.DVE)], 1)],
                            o[:, :, 127:128],
                        )
                        rot_toggle += 1
                        # store with circular row shift by r folded into DRAM
                        # offsets; dynamic-offset DMAs spread across engines
                        if q == 0:
                            nc.sync.dma_start(
                                out3[:, ds(rA, 64), :], orot[:, :, :]
                            )
                        else:
                            nc.scalar.dma_start(
                                out3[:, ds(rB, 63), :], orot[:, 0:63, :]
                            )
                            nc.gpsimd.dma_start(
                                out3[:, ds(rC, 1), :], orot[:, 63:64, :]
                            )
    return nc


def _get_nc():
    global _NC_CACHE
    if _NC_CACHE is None:
        _NC_CACHE = _build_nc()
    return _NC_CACHE


def _offsets_for(idx_pair):
    offs = np.zeros((1, 16), np.int32)
    for j, p in enumerate(idx_pair):
        p = int(p)
        r, c = p % 2, p // 2
        offs[0, 8 * j : 8 * j + 5] = (c, (127 + c) % 128, r, 64 + r, (127 + r) % 128)
    return offs


def kernel(inp, polyphase_indices, _trace=False):
    from concourse.bass_utils import run_bass_kernel_spmd

    inp = np.ascontiguousarray(np.asarray(inp), dtype=np.float32)
    idx = np.asarray(polyphase_indices).astype(np.int32).reshape(B)
    assert inp.shape == (B, C, N, N)

    in_maps = []
    for k in range(NCORES):
        in_maps.append(
            {
                "inp": np.ascontiguousarray(inp[NB * k : NB * (k + 1)]),
                "offs": _offsets_for(idx[NB * k : NB * (k + 1)]),
            }
        )

    nc = _get_nc()
    if not nc.is_finalized():
        nc.finalize()
    res = run_bass_kernel_spmd(
        nc, in_maps, core_ids=list(range(NCORES)), trace=_trace
    )
    out = np.concatenate([res.results[k]["out"] for k in range(NCORES)], axis=0)
    if _trace:
        kernel.last_results = res
    return out
